# revision 25
# baseline (speedup 1.0000x reference)
"""Trainium2 Bass kernel for nn_ALNNLayer (ALNN attention-like layer).

Reference computation (per batch b, ref-time k, step l, feature d):
    dist  = |T[b,l,d] - r_k|                      r_k = linspace(0,48,13)
    kern  = exp(-relu(alpha_k) * dist)
    inten = relu(X * kern) = relu(X) * kern       (kern > 0)
    pre   = wt0*X + wt1*DT + wt2*inten + wt3*M + 4*bt
    lat   = relu(pre)
    out[b,k,d] = relu( sum_l wv*lat + 200*bv[k,d] )

Strategy: data-parallel over batch (8 cores x 8 batches). Per core the
SBUF layout is [100 l-partitions, (j=l//100, b, d) free]; weights are
broadcast over b with stride-0 access patterns. Engine split:
  - VectorE: packed bf16 products (X*wt0 | DT*wt1 || M*wt3 | relu(X)*wt2)
    as two [100, 2048] ops, kern-apply (nonzero alpha_k only), wv
    multiply, final bias+relu epilogue
  - ScalarE: |T-r_k| and exp (needs only T+S, runs ahead), and the relu
    fused into the PSUM eviction
  - TensorE: term summation via identity matmuls accumulating in PSUM
    (bias first via a b-broadcast AP so accumulation starts before the
    products land), and the L-reduction via a k-column selector matmul
    emitted two positions behind so it never stalls the PE queue
Schedule: every DMA is a partition-major fully-contiguous blob (host
pre-packs); all transfers are issued upfront (in-flight DMA during the
steady state measurably slows the DVE ~20%), split across the two HWDGE
queues in need-order, within the 8 DMA completion-lane budget. Dummy
matmuls bridge the PE HAM clock gate through the DMA phase; a dummy
activation hoists the ACT table load. Zero-alpha and nonzero-alpha k's
are interleaved so ACT's dist/exp work spreads between PSUM evictions.
k's with relu(alpha_k) == 0 skip dist/exp/kern entirely (kern == 1); the
NEFF is compiled per alpha-sign-pattern, so this stays correct for any
inputs.
"""

import sys

for _p in ("/opt/trn_rl_repo", "/root/.axon_site/_ro/trn_rl_repo"):
    if _p not in sys.path:
        sys.path.append(_p)

import numpy as np
import ml_dtypes

import concourse.bass as bass
import concourse.bacc as bacc
import concourse.tile as tile
from concourse import mybir
from concourse.bass_utils import run_bass_kernel_spmd

B, L, D, K = 64, 200, 64, 13
NCORES = 8
BLOC = B // NCORES  # 8
PRIOR_HOURS = 48.0
REF_TIME = np.linspace(0.0, PRIOR_HOURS, K).astype(np.float32)

LP = 100            # l partitions
LJ = 2              # l super-tiles (l = j*LP + p)
NF = 4              # packed product features: X, DT, M, relu(X)

F32 = mybir.dt.float32
F16 = mybir.dt.float16
BF16 = mybir.dt.bfloat16
AX = mybir.AluOpType
AF = mybir.ActivationFunctionType
NPBF = ml_dtypes.bfloat16

N_WARM_MM = 20      # dummy matmuls to warm the PE HAM clock gate
NW0 = 1             # W positions in the first W blob
NWA = 6             # ... second blob (rest in the third)


def k_order(nonzero):
    """Zero-alpha k's interleaved with nonzero so ACT work spreads out."""
    zs = [k for k in range(K) if not nonzero[k]]
    nzs = [k for k in range(K) if nonzero[k]]
    order = []
    while zs or nzs:
        if zs:
            order.append(zs.pop(0))
        if nzs:
            order.append(nzs.pop(0))
    return order


def _bc(ap, nb=BLOC):
    """Insert a stride-0 b dim before the last free dim of an AP."""
    return bass.AP(
        tensor=ap.tensor, offset=ap.offset,
        ap=list(ap.ap[:-1]) + [[0, nb], ap.ap[-1]],
    )


def build_bass(nonzero):
    """nonzero: tuple of bool per k — whether relu(alpha_k) > 0."""
    nc = bacc.Bacc("TRN2", target_bir_lowering=False, debug=False)

    # all inputs partition-major, fully contiguous per partition
    # Da = (X, DT), Db = (M, relu(X)); [p, f, j, b, d] with l = j*LP + p
    Da_d = nc.declare_dram_parameter("Da", [LP, 2, LJ, BLOC, D], BF16, isOutput=False)
    Db_d = nc.declare_dram_parameter("Db", [LP, 2, LJ, BLOC, D], BF16, isOutput=False)
    T_d = nc.declare_dram_parameter("T4", [LP, LJ, BLOC, D], F16, isOutput=False)
    # per-position weights (k's pre-permuted into consumption order):
    # [p, pos, 6, j, d] with f-order (wt0, wt1, wt3, wt2, 4bt, wv)
    W0_d = nc.declare_dram_parameter("W0", [LP, NW0, NF + 2, LJ, D], BF16, isOutput=False)
    Wa_d = nc.declare_dram_parameter("Wa", [LP, NWA, NF + 2, LJ, D], BF16, isOutput=False)
    Wb_d = nc.declare_dram_parameter(
        "Wb", [LP, K - NW0 - NWA, NF + 2, LJ, D], BF16, isOutput=False)
    # CF: [128, 26+64] f32 = S (exp scales | abs biases) | 200*b_v padded
    CF_d = nc.declare_dram_parameter("CF", [128, 2 * K + D], F32, isOutput=False)
    # CB: [128, 100+169] bf16 = eye(100) padded | k-column selector blocks
    CB_d = nc.declare_dram_parameter("CB", [128, LP + K * K], BF16, isOutput=False)
    out_d = nc.declare_dram_parameter("out", [BLOC, K, D], F32, isOutput=True)

    order = k_order(nonzero)

    from contextlib import ExitStack

    with tile.TileContext(nc) as tc, ExitStack() as ctx:
        const = ctx.enter_context(tc.tile_pool(name="const", bufs=1))
        tmp = ctx.enter_context(tc.tile_pool(name="tmp", bufs=3))
        psum = ctx.enter_context(tc.tile_pool(name="psum", bufs=3, space="PSUM"))
        psum1 = ctx.enter_context(tc.tile_pool(name="psum1", bufs=1, space="PSUM"))
        psumw = ctx.enter_context(tc.tile_pool(name="psumw", bufs=1, space="PSUM"))

        # ---- DMAs: HWDGE only, all upfront, split across both queues ----
        Dp = const.tile([LP, NF, LJ, BLOC, D], BF16, tag="Dp")
        W0t = const.tile([LP, NW0, NF + 2, LJ, D], BF16, tag="W0t")
        Wat = const.tile([LP, NWA, NF + 2, LJ, D], BF16, tag="Wat")
        Wbt = const.tile([LP, K - NW0 - NWA, NF + 2, LJ, D], BF16, tag="Wbt")
        CF = const.tile([128, 2 * K + D], F32, tag="CF")
        CB = const.tile([128, LP + K * K], BF16, tag="CB")
        Tt = const.tile([LP, LJ, BLOC, D], F16, tag="T")

        # sync queue: X|DT, first W, S/BV consts, W positions 1-6
        nc.sync.dma_start(out=Dp[:, 0:2], in_=Da_d[:])
        nc.sync.dma_start(out=W0t[:], in_=W0_d[:])
        nc.sync.dma_start(out=CF[:], in_=CF_d[:])
        nc.sync.dma_start(out=Wat[:], in_=Wa_d[:])
        # scalar queue: M|relu(X), eye/selector, T, W positions 7-12
        nc.scalar.dma_start(out=Dp[:, 2:4], in_=Db_d[:])
        nc.scalar.dma_start(out=CB[:], in_=CB_d[:])
        nc.scalar.dma_start(out=Tt[:], in_=T_d[:])
        nc.scalar.dma_start(out=Wbt[:], in_=Wb_d[:])

        S_sb = CF[:, : 2 * K]
        BV_sb = CF[:K, 2 * K :]
        EYE = CB[:LP, :LP]
        E_sb = CB[:, LP:]

        def wslot(i):
            if i < NW0:
                return W0t[:, i]
            if i < NW0 + NWA:
                return Wat[:, i - NW0]
            return Wbt[:, i - NW0 - NWA]

        # ---- PE warm-up + ACT table-load hoist during the DMA phase ----
        warm = const.tile([128, 640], BF16, tag="warm")
        nc.vector.memset(warm[:], 0.0)
        pw = psumw.tile([128, 512], F32, tag="pw")
        for _ in range(N_WARM_MM):
            nc.tensor.matmul(pw[:], warm[:, :128], warm[:, 128:], start=True, stop=True)
        dummy_act = const.tile([1, 8], F32, tag="dact")
        nc.scalar.activation(dummy_act[:], warm[:1, :8], AF.Exp)

        # ---- per-k stages ----
        kerns = {}

        def emit_distexp(k):
            dist = tmp.tile([LP, LJ, BLOC, D], F32, tag="dist")
            nc.scalar.activation(
                dist[:], Tt[:], AF.Abs,
                bias=S_sb[:LP, K + k : K + k + 1], scale=1.0,
            )
            kern = const.tile([LP, LJ, BLOC, D], BF16, tag=f"kern{k}")
            nc.scalar.activation(kern[:], dist[:], AF.Exp, scale=S_sb[:LP, k : k + 1])
            kerns[k] = kern

        osb = const.tile([K, BLOC, D], F32)
        po = psum1.tile([K, BLOC, D], F32)  # L-sums, one bank, rows = k

        Sps, Qs, zts, pres = {}, {}, {}, {}

        def stage_product(i):
            k = order[i]
            if nonzero[k]:
                emit_distexp(k)
            w = wslot(i)
            Sp = tmp.tile([LP, NF, LJ, BLOC, D], BF16, tag="Sp")
            for f0 in (0, 2):
                wap = bass.AP(
                    tensor=w.tensor,
                    offset=w.offset + f0 * LJ * D,
                    ap=[w.ap[0], [LJ * D, 2], [D, LJ], [0, BLOC], [1, D]],
                )
                nc.vector.tensor_tensor(
                    Sp[:, f0 : f0 + 2], Dp[:, f0 : f0 + 2], wap, AX.mult
                )
            Sps[i] = Sp
            if nonzero[k]:
                Q = tmp.tile([LP, LJ, BLOC, D], BF16, tag="Q")
                nc.vector.tensor_tensor(Q[:], Sp[:, 3], kerns[k][:], AX.mult)
                Qs[i] = Q

        def stage_sel(i):
            # selector matmuls for position i (z ready ~2 positions ago)
            k = order[i]
            for j in range(LJ):
                nc.tensor.matmul(
                    po[:, :, :],
                    E_sb[:LP, k * K : (k + 1) * K],
                    zts[i][:, j],
                    start=(i == 0 and j == 0),
                    stop=(i == K - 1 and j == LJ - 1),
                )

        def stage_mms(i):
            if i >= 2:
                stage_sel(i - 2)
            Sp = Sps[i]
            qterm = Qs.get(i)
            w = wslot(i)
            # bias first: only needs W (starts accumulation before products)
            terms = [_bc(w[:, NF])]
            terms += [Sp[:, 0], Sp[:, 1], Sp[:, 2]]
            terms.append(qterm[:] if qterm is not None else Sp[:, 3])
            pre = psum.tile([LP, LJ, BLOC, D], F32, tag="pre")
            for ti, t in enumerate(terms):
                for j in range(LJ):
                    nc.tensor.matmul(
                        pre[:, j], EYE[:LP, :LP], t[:, j],
                        start=(ti == 0), stop=(ti == len(terms) - 1),
                    )
            return pre

        def stage_evict(i):
            pre = pres[i]
            w = wslot(i)
            lat = tmp.tile([LP, LJ, BLOC, D], BF16, tag="lat")
            nc.scalar.activation(lat[:], pre[:], AF.Relu)
            z = tmp.tile([LP, LJ, BLOC, D], BF16, tag="z")
            nc.vector.tensor_tensor(z[:], lat[:], _bc(w[:, NF + 1]), AX.mult)
            zts[i] = z

        stage_product(0)
        stage_product(1)
        for i in range(K):
            if i + 2 < K:
                stage_product(i + 2)
            pres[i] = stage_mms(i)
            if i >= 1:
                stage_evict(i - 1)
        stage_evict(K - 1)
        stage_sel(K - 2)
        stage_sel(K - 1)

        # ---- epilogue: out = relu(po + 200*bv) ----
        nc.vector.tensor_tensor(osb[:], po[:], _bc(BV_sb[:]), AX.add)
        nc.vector.tensor_scalar_max(osb[:], osb[:], 0.0)
        nc.scalar.dma_start(out=out_d[:].rearrange("b k d -> k b d"), in_=osb[:])

    nc.compile()
    return nc


_NC_CACHE = {}


def _get_nc(nonzero):
    key = tuple(nonzero)
    if key not in _NC_CACHE:
        _NC_CACHE[key] = build_bass(key)
    return _NC_CACHE[key]


def make_in_maps(X, T, M, DT, alpha, w_v, w_t, b_v, b_t):
    X = np.asarray(X, np.float32)
    T = np.asarray(T, np.float32)
    M = np.asarray(M, np.float32)
    DT = np.asarray(DT, np.float32)
    w_t = np.asarray(w_t, np.float32)
    w_v = np.asarray(w_v, np.float32)
    b_t = np.asarray(b_t, np.float32)
    b_v = np.asarray(b_v, np.float32)
    alpha = np.asarray(alpha, np.float32).reshape(K)

    nonzero = tuple(bool(a > 0) for a in alpha)
    order = k_order(nonzero)

    # weight pack: [K, L, 6, D] with f-order (wt0, wt1, wt3, wt2, 4bt, wv)
    W = np.empty((K, L, NF + 2, D), np.float32)
    W[:, :, 0] = w_t[:, :, :, 0]
    W[:, :, 1] = w_t[:, :, :, 1]
    W[:, :, 2] = w_t[:, :, :, 3]
    W[:, :, 3] = w_t[:, :, :, 2]
    W[:, :, 4] = 4.0 * b_t[:, :, :, 0]
    W[:, :, 5] = w_v
    # -> [LP, K, 6, LJ, D], partition-major, k's in consumption order
    W = W.reshape(K, LJ, LP, NF + 2, D).transpose(2, 0, 3, 1, 4)[:, list(order)]
    W = np.ascontiguousarray(W).astype(NPBF)
    W0 = np.ascontiguousarray(W[:, :NW0])
    Wa = np.ascontiguousarray(W[:, NW0 : NW0 + NWA])
    Wb = np.ascontiguousarray(W[:, NW0 + NWA :])

    # CF: [128, 26+64] f32 = S | 200*b_v (padded to 128 rows)
    CF = np.zeros((128, 2 * K + D), np.float32)
    CF[:, :K] = -np.maximum(alpha.reshape(1, K), 0.0)
    CF[:, K : 2 * K] = -REF_TIME.reshape(1, K)
    CF[:K, 2 * K :] = float(L) * b_v[:, 0, :]
    # CB: [128, 100+169] bf16 = eye(100) | selector columns
    CB = np.zeros((128, LP + K * K), np.float32)
    CB[:LP, :LP] = np.eye(LP)
    for k in range(K):
        CB[:, LP + k * K + k] = 1.0
    CB = CB.astype(NPBF)

    def trp(A):
        # [BLOC, L, D] -> [LP, LJ, BLOC, D], partition-major
        return np.ascontiguousarray(
            A.reshape(BLOC, LJ, LP, D).transpose(2, 1, 0, 3)
        )

    in_maps = []
    for c in range(NCORES):
        b0 = c * BLOC
        bs = slice(b0, b0 + BLOC)
        Da = np.stack([trp(X[bs].astype(NPBF)), trp(DT[bs].astype(NPBF))], axis=1)
        Db = np.stack(
            [trp(M[bs].astype(NPBF)), trp(np.maximum(X[bs], 0.0).astype(NPBF))],
            axis=1,
        )
        in_maps.append(
            {
                "Da": np.ascontiguousarray(Da),
                "Db": np.ascontiguousarray(Db),
                "T4": trp(T[bs]).astype(np.float16),
                "W0": W0,
                "Wa": Wa,
                "Wb": Wb,
                "CF": CF,
                "CB": CB,
            }
        )
    return in_maps, nonzero


def kernel(X, T, M, DT, alpha, w_v, w_t, b_v, b_t):
    in_maps, nonzero = make_in_maps(X, T, M, DT, alpha, w_v, w_t, b_v, b_t)
    nc = _get_nc(nonzero)
    res = run_bass_kernel_spmd(nc, in_maps, core_ids=list(range(NCORES)))
    out = np.concatenate([res.results[c]["out"] for c in range(NCORES)], axis=0)
    return out.astype(np.float32)


# revision 27
# speedup vs baseline: 1.0785x; 1.0785x over previous
"""Trainium2 Bass kernel for nn_ALNNLayer (ALNN attention-like layer).

Reference computation (per batch b, ref-time k, step l, feature d):
    dist  = |T[b,l,d] - r_k|                      r_k = linspace(0,48,13)
    kern  = exp(-relu(alpha_k) * dist)
    inten = relu(X * kern) = relu(X) * kern       (kern > 0)
    pre   = wt0*X + wt1*DT + wt2*inten + wt3*M + 4*bt
    lat   = relu(pre)
    out[b,k,d] = relu( sum_l wv*lat + 200*bv[k,d] )

Strategy: data-parallel over batch (8 cores x 8 batches). Per core the
SBUF layout is [100 l-partitions, (j=l//100, b, d) free]; weights are
broadcast over b with stride-0 access patterns. Engine split:
  - VectorE: packed bf16 products (X*wt0 | DT*wt1 || M*wt3 | relu(X)*wt2)
    as two [100, 2048] ops, kern-apply (nonzero alpha_k only), wv
    multiply, final bias+relu epilogue
  - ScalarE: |T-r_k| and exp (needs only T+S, runs ahead), and the relu
    fused into the PSUM eviction
  - TensorE: term summation via identity matmuls accumulating in PSUM
    (bias first via a b-broadcast AP so accumulation starts before the
    products land), and the L-reduction via a k-column selector matmul
    emitted two positions behind so it never stalls the PE queue
Schedule: every DMA is a partition-major fully-contiguous blob (host
pre-packs); all transfers are issued upfront (in-flight DMA during the
steady state measurably slows the DVE ~20%), split across the two HWDGE
queues in need-order, within the 8 DMA completion-lane budget. Dummy
matmuls bridge the PE HAM clock gate through the DMA phase; a dummy
activation hoists the ACT table load. Zero-alpha and nonzero-alpha k's
are interleaved so ACT's dist/exp work spreads between PSUM evictions.
k's with relu(alpha_k) == 0 skip dist/exp/kern entirely (kern == 1); the
NEFF is compiled per alpha-sign-pattern, so this stays correct for any
inputs.
"""

import sys

for _p in ("/opt/trn_rl_repo", "/root/.axon_site/_ro/trn_rl_repo"):
    if _p not in sys.path:
        sys.path.append(_p)

import numpy as np
import ml_dtypes

import concourse.bass as bass
import concourse.bacc as bacc
import concourse.tile as tile
from concourse import mybir
from concourse.bass_utils import run_bass_kernel_spmd

B, L, D, K = 64, 200, 64, 13
NCORES = 8
BLOC = B // NCORES  # 8
PRIOR_HOURS = 48.0
REF_TIME = np.linspace(0.0, PRIOR_HOURS, K).astype(np.float32)

LP = 100            # l partitions
LJ = 2              # l super-tiles (l = j*LP + p)
NF = 4              # packed product features: X, DT, M, relu(X)

F32 = mybir.dt.float32
F16 = mybir.dt.float16
BF16 = mybir.dt.bfloat16
AX = mybir.AluOpType
AF = mybir.ActivationFunctionType
NPBF = ml_dtypes.bfloat16

N_WARM_MM = 20      # dummy matmuls to warm the PE HAM clock gate
NW0 = 3             # W positions in the first W blob
NWA = 4             # ... second blob (rest in the third)


def k_order(nonzero):
    """Zero-alpha k's interleaved with nonzero so ACT work spreads out."""
    zs = [k for k in range(K) if not nonzero[k]]
    nzs = [k for k in range(K) if nonzero[k]]
    order = []
    while zs or nzs:
        if zs:
            order.append(zs.pop(0))
        if nzs:
            order.append(nzs.pop(0))
    return order


def _bc(ap, nb=BLOC):
    """Insert a stride-0 b dim before the last free dim of an AP."""
    return bass.AP(
        tensor=ap.tensor, offset=ap.offset,
        ap=list(ap.ap[:-1]) + [[0, nb], ap.ap[-1]],
    )


def build_bass(nonzero):
    """nonzero: tuple of bool per k — whether relu(alpha_k) > 0."""
    nc = bacc.Bacc("TRN2", target_bir_lowering=False, debug=False)

    # all inputs partition-major, fully contiguous per partition
    # Da = (X, DT), Db = (M, relu(X)); [p, f, j, b, d] with l = j*LP + p
    Da_d = nc.declare_dram_parameter("Da", [LP, 2, LJ, BLOC, D], BF16, isOutput=False)
    Db_d = nc.declare_dram_parameter("Db", [LP, 2, LJ, BLOC, D], BF16, isOutput=False)
    T_d = nc.declare_dram_parameter("T4", [LP, LJ, BLOC, D], F16, isOutput=False)
    # per-position weights (k's pre-permuted into consumption order):
    # [p, pos, 6, j, d] with f-order (wt0, wt1, wt3, wt2, 4bt, wv)
    W0_d = nc.declare_dram_parameter("W0", [LP, NW0, NF + 2, LJ, D], BF16, isOutput=False)
    Wa_d = nc.declare_dram_parameter("Wa", [LP, NWA, NF + 2, LJ, D], BF16, isOutput=False)
    Wb_d = nc.declare_dram_parameter(
        "Wb", [LP, K - NW0 - NWA, NF + 2, LJ, D], BF16, isOutput=False)
    # CF: [128, 26+64] f32 = S (exp scales | abs biases) | 200*b_v padded
    CF_d = nc.declare_dram_parameter("CF", [128, 2 * K + D], F32, isOutput=False)
    # CB: [128, 100+169] bf16 = eye(100) padded | k-column selector blocks
    CB_d = nc.declare_dram_parameter("CB", [128, LP + K * K], BF16, isOutput=False)
    out_d = nc.declare_dram_parameter("out", [BLOC, K, D], F32, isOutput=True)

    order = k_order(nonzero)

    from contextlib import ExitStack

    with tile.TileContext(nc) as tc, ExitStack() as ctx:
        const = ctx.enter_context(tc.tile_pool(name="const", bufs=1))
        tmp = ctx.enter_context(tc.tile_pool(name="tmp", bufs=3))
        psum = ctx.enter_context(tc.tile_pool(name="psum", bufs=3, space="PSUM"))
        psum1 = ctx.enter_context(tc.tile_pool(name="psum1", bufs=1, space="PSUM"))
        psumw = ctx.enter_context(tc.tile_pool(name="psumw", bufs=1, space="PSUM"))

        # ---- DMAs: HWDGE only, all upfront, split across both queues ----
        Dp = const.tile([LP, NF, LJ, BLOC, D], BF16, tag="Dp")
        W0t = const.tile([LP, NW0, NF + 2, LJ, D], BF16, tag="W0t")
        Wat = const.tile([LP, NWA, NF + 2, LJ, D], BF16, tag="Wat")
        Wbt = const.tile([LP, K - NW0 - NWA, NF + 2, LJ, D], BF16, tag="Wbt")
        CF = const.tile([128, 2 * K + D], F32, tag="CF")
        CB = const.tile([128, LP + K * K], BF16, tag="CB")
        Tt = const.tile([LP, LJ, BLOC, D], F16, tag="T")

        # sync queue: X|DT, W positions 0-2, S/BV consts, W positions 7-12
        nc.sync.dma_start(out=Dp[:, 0:2], in_=Da_d[:])
        nc.sync.dma_start(out=W0t[:], in_=W0_d[:])
        nc.sync.dma_start(out=CF[:], in_=CF_d[:])
        nc.sync.dma_start(out=Wbt[:], in_=Wb_d[:])
        # scalar queue: M|relu(X), eye/selector, T, W positions 3-6
        nc.scalar.dma_start(out=Dp[:, 2:4], in_=Db_d[:])
        nc.scalar.dma_start(out=CB[:], in_=CB_d[:])
        nc.scalar.dma_start(out=Tt[:], in_=T_d[:])
        nc.scalar.dma_start(out=Wat[:], in_=Wa_d[:])

        S_sb = CF[:, : 2 * K]
        BV_sb = CF[:K, 2 * K :]
        EYE = CB[:LP, :LP]
        E_sb = CB[:, LP:]

        def wslot(i):
            if i < NW0:
                return W0t[:, i]
            if i < NW0 + NWA:
                return Wat[:, i - NW0]
            return Wbt[:, i - NW0 - NWA]

        # ---- PE warm-up + ACT table-load hoist during the DMA phase ----
        warm = const.tile([128, 640], BF16, tag="warm")
        nc.vector.memset(warm[:], 0.0)
        pw = psumw.tile([128, 512], F32, tag="pw")
        for _ in range(N_WARM_MM):
            nc.tensor.matmul(pw[:], warm[:, :128], warm[:, 128:], start=True, stop=True)
        dummy_act = const.tile([1, 8], F32, tag="dact")
        nc.scalar.activation(dummy_act[:], warm[:1, :8], AF.Exp)

        # ---- per-k stages ----
        kerns = {}

        def emit_distexp(k):
            dist = tmp.tile([LP, LJ, BLOC, D], F32, tag="dist")
            nc.scalar.activation(
                dist[:], Tt[:], AF.Abs,
                bias=S_sb[:LP, K + k : K + k + 1], scale=1.0,
            )
            kern = const.tile([LP, LJ, BLOC, D], BF16, tag=f"kern{k}")
            nc.scalar.activation(kern[:], dist[:], AF.Exp, scale=S_sb[:LP, k : k + 1])
            kerns[k] = kern

        osb = const.tile([K, BLOC, D], F32)
        po = psum1.tile([K, BLOC, D], F32)  # L-sums, one bank, rows = k

        Sps, Qs, zts, pres = {}, {}, {}, {}

        def stage_product(i):
            k = order[i]
            if nonzero[k]:
                emit_distexp(k)
            w = wslot(i)
            Sp = tmp.tile([LP, NF, LJ, BLOC, D], BF16, tag="Sp")
            for f0 in (0, 2):
                wap = bass.AP(
                    tensor=w.tensor,
                    offset=w.offset + f0 * LJ * D,
                    ap=[w.ap[0], [LJ * D, 2], [D, LJ], [0, BLOC], [1, D]],
                )
                nc.vector.tensor_tensor(
                    Sp[:, f0 : f0 + 2], Dp[:, f0 : f0 + 2], wap, AX.mult
                )
            Sps[i] = Sp
            if nonzero[k]:
                Q = tmp.tile([LP, LJ, BLOC, D], BF16, tag="Q")
                nc.vector.tensor_tensor(Q[:], Sp[:, 3], kerns[k][:], AX.mult)
                Qs[i] = Q

        def stage_sel(i):
            # selector matmuls for position i (z ready ~2 positions ago)
            k = order[i]
            for j in range(LJ):
                nc.tensor.matmul(
                    po[:, :, :],
                    E_sb[:LP, k * K : (k + 1) * K],
                    zts[i][:, j],
                    start=(i == 0 and j == 0),
                    stop=(i == K - 1 and j == LJ - 1),
                )

        def stage_mms(i):
            if i >= 2:
                stage_sel(i - 2)
            Sp = Sps[i]
            qterm = Qs.get(i)
            w = wslot(i)
            # bias first: only needs W (starts accumulation before products)
            terms = [_bc(w[:, NF])]
            terms += [Sp[:, 0], Sp[:, 1], Sp[:, 2]]
            terms.append(qterm[:] if qterm is not None else Sp[:, 3])
            pre = psum.tile([LP, LJ, BLOC, D], F32, tag="pre")
            for ti, t in enumerate(terms):
                for j in range(LJ):
                    nc.tensor.matmul(
                        pre[:, j], EYE[:LP, :LP], t[:, j],
                        start=(ti == 0), stop=(ti == len(terms) - 1),
                    )
            return pre

        def stage_evict(i):
            pre = pres[i]
            w = wslot(i)
            lat = tmp.tile([LP, LJ, BLOC, D], BF16, tag="lat")
            nc.scalar.activation(lat[:], pre[:], AF.Relu)
            z = tmp.tile([LP, LJ, BLOC, D], BF16, tag="z")
            nc.vector.tensor_tensor(z[:], lat[:], _bc(w[:, NF + 1]), AX.mult)
            zts[i] = z

        stage_product(0)
        stage_product(1)
        for i in range(K):
            if i + 2 < K:
                stage_product(i + 2)
            pres[i] = stage_mms(i)
            if i >= 1:
                stage_evict(i - 1)
        stage_evict(K - 1)
        stage_sel(K - 2)
        stage_sel(K - 1)

        # ---- epilogue: out = relu(po + 200*bv) ----
        nc.vector.tensor_tensor(osb[:], po[:], _bc(BV_sb[:]), AX.add)
        nc.vector.tensor_scalar_max(osb[:], osb[:], 0.0)
        nc.scalar.dma_start(out=out_d[:].rearrange("b k d -> k b d"), in_=osb[:])

    nc.compile()
    return nc


_NC_CACHE = {}


def _get_nc(nonzero):
    key = tuple(nonzero)
    if key not in _NC_CACHE:
        _NC_CACHE[key] = build_bass(key)
    return _NC_CACHE[key]


def make_in_maps(X, T, M, DT, alpha, w_v, w_t, b_v, b_t):
    X = np.asarray(X, np.float32)
    T = np.asarray(T, np.float32)
    M = np.asarray(M, np.float32)
    DT = np.asarray(DT, np.float32)
    w_t = np.asarray(w_t, np.float32)
    w_v = np.asarray(w_v, np.float32)
    b_t = np.asarray(b_t, np.float32)
    b_v = np.asarray(b_v, np.float32)
    alpha = np.asarray(alpha, np.float32).reshape(K)

    nonzero = tuple(bool(a > 0) for a in alpha)
    order = k_order(nonzero)

    # weight pack: [K, L, 6, D] with f-order (wt0, wt1, wt3, wt2, 4bt, wv)
    W = np.empty((K, L, NF + 2, D), np.float32)
    W[:, :, 0] = w_t[:, :, :, 0]
    W[:, :, 1] = w_t[:, :, :, 1]
    W[:, :, 2] = w_t[:, :, :, 3]
    W[:, :, 3] = w_t[:, :, :, 2]
    W[:, :, 4] = 4.0 * b_t[:, :, :, 0]
    W[:, :, 5] = w_v
    # -> [LP, K, 6, LJ, D], partition-major, k's in consumption order
    W = W.reshape(K, LJ, LP, NF + 2, D).transpose(2, 0, 3, 1, 4)[:, list(order)]
    W = np.ascontiguousarray(W).astype(NPBF)
    W0 = np.ascontiguousarray(W[:, :NW0])
    Wa = np.ascontiguousarray(W[:, NW0 : NW0 + NWA])
    Wb = np.ascontiguousarray(W[:, NW0 + NWA :])

    # CF: [128, 26+64] f32 = S | 200*b_v (padded to 128 rows)
    CF = np.zeros((128, 2 * K + D), np.float32)
    CF[:, :K] = -np.maximum(alpha.reshape(1, K), 0.0)
    CF[:, K : 2 * K] = -REF_TIME.reshape(1, K)
    CF[:K, 2 * K :] = float(L) * b_v[:, 0, :]
    # CB: [128, 100+169] bf16 = eye(100) | selector columns
    CB = np.zeros((128, LP + K * K), np.float32)
    CB[:LP, :LP] = np.eye(LP)
    for k in range(K):
        CB[:, LP + k * K + k] = 1.0
    CB = CB.astype(NPBF)

    def trp(A):
        # [BLOC, L, D] -> [LP, LJ, BLOC, D], partition-major
        return np.ascontiguousarray(
            A.reshape(BLOC, LJ, LP, D).transpose(2, 1, 0, 3)
        )

    in_maps = []
    for c in range(NCORES):
        b0 = c * BLOC
        bs = slice(b0, b0 + BLOC)
        Da = np.stack([trp(X[bs].astype(NPBF)), trp(DT[bs].astype(NPBF))], axis=1)
        Db = np.stack(
            [trp(M[bs].astype(NPBF)), trp(np.maximum(X[bs], 0.0).astype(NPBF))],
            axis=1,
        )
        in_maps.append(
            {
                "Da": np.ascontiguousarray(Da),
                "Db": np.ascontiguousarray(Db),
                "T4": trp(T[bs]).astype(np.float16),
                "W0": W0,
                "Wa": Wa,
                "Wb": Wb,
                "CF": CF,
                "CB": CB,
            }
        )
    return in_maps, nonzero


def kernel(X, T, M, DT, alpha, w_v, w_t, b_v, b_t):
    in_maps, nonzero = make_in_maps(X, T, M, DT, alpha, w_v, w_t, b_v, b_t)
    nc = _get_nc(nonzero)
    res = run_bass_kernel_spmd(nc, in_maps, core_ids=list(range(NCORES)))
    out = np.concatenate([res.results[c]["out"] for c in range(NCORES)], axis=0)
    return out.astype(np.float32)
